# revision 39
# baseline (speedup 1.0000x reference)
"""Trainium2 Bass kernel for nn_EncodingModule2d (vq_codebook).

Pipeline per batch item (pure data parallel, 1 item per NeuronCore, 8 cores):
  stem:   s = conv_w @ x  (1x1 conv as 256x256 matmul over 4096 positions)
          y = relu(BN2(s))                          -- BN folded into weights on host
  vq:     dist2[n,k] = |y_n|^2 - 2<y_n, c_k> + |c_k|^2
          a = softmax_k(scales_k * dist2)
          agg[k,:] = sum_n a[n,k] (y_n - c_k)
  post:   z = mean_k relu(BN1(agg))                 -- BN folded on host
          g = sigmoid(head_w @ z + head_b)
  out:    relu(x + x * g) = relu(x * (1 + g))

The kernel computes the stem in BOTH (d,n) and (n,d) layouts directly from x
(two matmul orientations) because the distance matmul contracts over d while
the aggregation matmul contracts over n; this costs the same PE time as one
stem plus PE transposes but avoids ~20us of PSUM->SBUF copy traffic.

dtype strategy: float32r (1 cyc/row on the PE when N>=256, vs 4 for float32)
for the stem and aggregation matmuls; plain float32 for the N=32 distance
matmuls where fp32r has no speed advantage anyway (and would force rounding
of y). fp32r matmuls require even N, hence the 258-wide aggregation rhs
(256 y columns + ones column + dummy pad column).
"""

import os
import sys

for _p in ("/opt/trn_rl_repo",):
    if _p not in sys.path and os.path.isdir(_p):
        sys.path.insert(0, _p)

from contextlib import ExitStack

import numpy as np

import concourse.bass as bass
import concourse.tile as tile
from concourse import bacc, mybir
from concourse.bass_utils import run_bass_kernel_spmd
from concourse.masks import make_identity

F32 = mybir.dt.float32
F32R = mybir.dt.float32r
AF = mybir.ActivationFunctionType
ALU = mybir.AluOpType

B, D, H, W, K = 8, 256, 64, 64, 32
HW = H * W          # 4096 spatial positions
NB = D // 128       # 2 channel blocks of 128
NS = HW // 512      # 8 n-slices of 512
NCH = HW // 128     # 32 n-chunks of 128
CW = D + 2          # y_nd chunk width: 256 y + ones + pad (fp32r needs even N)
EPS = 1e-5
N_CORES = 8


def _strided_cols(t, start, step, count, width):
    """AP over columns [start + i*step : start + i*step + width) of a 2D tile."""
    a = t[:, start : start + 1]
    return bass.AP(tensor=a.tensor, offset=a.offset, ap=[a.ap[0], [step, count], [1, width]])


def _build_program(has_bias2):
    nc = bacc.Bacc("TRN2", target_bir_lowering=False, debug=False, num_devices=N_CORES)

    x_d = nc.dram_tensor("x", [D, HW], F32R, kind="ExternalInput").ap()
    # packed constants: fewer DMA triggers (each costs ~0.7us of SEQ time)
    # packr: [wT | ct2 | identity(128) in c-block 0 rows]
    pr_d = nc.dram_tensor("packr", [D, D + K + 128], F32R, kind="ExternalInput").ap()
    pf_d = nc.dram_tensor("packf", [D, D + 4], F32, kind="ExternalInput").ap()    # [hwT | chv]
    sm_d = nc.dram_tensor("small", [K, D + 1], F32, kind="ExternalInput").ap()    # [ckd | scc]
    ssc_d = nc.dram_tensor("ssc", [1, K], F32R, kind="ExternalInput").ap()
    one2_d = nc.dram_tensor("one2", [1, 2], F32R, kind="ExternalInput").ap()
    b2r_d = nc.dram_tensor("b2r", [2, D], F32R, kind="ExternalInput").ap()  # [ones, bias2]
    out_d = nc.dram_tensor("out", [D, HW], F32, kind="ExternalOutput").ap()

    x3 = x_d.rearrange("(c p) n -> p c n", p=128)

    with tile.TileContext(nc) as tc, ExitStack() as ctx:
        sb = ctx.enter_context(tc.tile_pool(name="sb", bufs=1))

        # ---- loads: first x slice, then weights, then the rest ---------
        x_sb = sb.tile([128, NB, HW], F32R)
        packr = sb.tile([128, NB, D + K + 128], F32R)
        packf = sb.tile([128, NB, D + 4], F32)
        small = sb.tile([32, D + 1], F32)
        srep = sb.tile([128, K], F32R)           # scales replicated over partitions
        one2 = sb.tile([128, 2], F32R)           # fp32r ones (memset can't do f32r)
        b2row = sb.tile([2, D], F32R)            # [ones row, bias2 row] (stem A bias)

        # x: contiguous per-c-block pieces, c0 on the sync queue and c1 on
        # the scalar queue so both flow in parallel and slice 0 lands early
        qeng = [nc.sync, nc.scalar]
        pieces = [(0, 512), (512, 1536), (1536, 2560), (2560, 4096)]
        for q, (lo, hi) in enumerate(pieces):
            cs = slice(lo, hi)
            for c in range(NB):
                qeng[c].dma_start(x_sb[:, c, cs], x_d[c * 128 : (c + 1) * 128, cs])
            if q == 0:
                nc.sync.dma_start(packr[:], pr_d.rearrange("(c p) m -> p c m", p=128))
        nc.scalar.dma_start(packf[:], pf_d.rearrange("(c p) m -> p c m", p=128))
        nc.sync.dma_start(small[:], sm_d)
        nc.scalar.dma_start(srep[:], ssc_d.partition_broadcast(128))
        nc.sync.dma_start(one2[:], one2_d.partition_broadcast(128))
        if has_bias2:
            nc.sync.dma_start(b2row[:], b2r_d)

        wT = packr[:, :, 0:D]                    # wT[c,:,o] per c-block
        ct2 = packr[:, :, D : D + K]             # -2*scales[k]*centers[k,d]
        ident128 = packr[:, 0, D + K : D + K + 128]   # 128x128 identity (f32r)
        hwT = packf[:, :, 0:D]                   # head_w.T / K
        chv = packf[:, :, D : D + 4]             # [bias2, s1, bb1, -head_b]
        ckd = small[:, 0:D]                      # centers (k,d)
        sc2col = small[:, D : D + 1]             # scales[k]*|c_k|^2 (bias column)

        ident = sb.tile([32, 32], F32)
        make_identity(nc, ident[:])

        # warm the exp table on ACT early (hidden under the x DMA)
        warm = sb.tile([128, 1], F32)
        nc.vector.memset(warm[:], 0.0)
        nc.scalar.activation(warm[:], warm[:], AF.Exp)

        # ---- big intermediates ----------------------------------------
        y_dn = sb.tile([128, NB, HW], F32R)      # relu(W'x): d on partitions
        y_nd = sb.tile([128, NCH * CW], F32R)    # per chunk: 256 y cols + [1, 1]
        ysq = sb.tile([128, NB, HW], F32R)       # y_dn^2
        lkn = sb.tile([32, HW], F32)             # logits in (k, n) layout
        esub = sb.tile([128, NCH * K], F32)      # logits - max
        e_sb = sb.tile([128, NCH * K], F32)      # exp(...)
        a_sb = sb.tile([128, NCH * K], F32R)     # softmax weights
        out_sb = sb.tile([128, NB, HW], F32)

        # ones + pad columns of y_nd (DVE copy from f32r const)
        nc.vector.tensor_copy(
            _strided_cols(y_nd, D, CW, NCH, 2),
            one2[:].rearrange("p (u k) -> p u k", u=1).broadcast_to((128, NCH, 2)))

        with ExitStack() as stem_ctx:
            psB = stem_ctx.enter_context(tc.tile_pool(name="psB", bufs=2, space="PSUM"))
            psA = stem_ctx.enter_context(tc.tile_pool(name="psA", bufs=2, space="PSUM"))
            psK = stem_ctx.enter_context(tc.tile_pool(name="psK", bufs=2, space="PSUM"))
            psL = stem_ctx.enter_context(tc.tile_pool(name="psL", bufs=1, space="PSUM"))

            logits_ps = [psL.tile([128, 512], F32, name=f"logits{i}") for i in range(2)]
            maxt = sb.tile([128, NCH], F32)
            sumt = sb.tile([128, NCH], F32)
            rcp = sb.tile([128, NCH], F32)

            # HAM warm-up: keep the PE busy on dummy transposes of the small
            # identity while x streams in, so the clock gate is at 8/8 when
            # the real matmuls start. Results land in a logits_ps corner that
            # the real chunk transposes later overwrite (start=True).
            for i in range(32):
                nc.tensor.transpose(logits_ps[i % 2][0:32, 128:160],
                                    ident[:], ident[:])

            def emit_softmax(g0, gn):
                t = g0 // 16
                gs = slice(g0, g0 + gn)
                cs = slice(g0 * K, (g0 + gn) * K)
                lcs = slice((g0 % 16) * K, ((g0 % 16) + gn) * K)
                lp3 = logits_ps[t][:, lcs].rearrange("p (g k) -> p g k", g=gn)
                nc.vector.tensor_reduce(out=maxt[:, gs], in_=lp3,
                                        axis=mybir.AxisListType.X, op=ALU.max)
                mb = maxt[:, gs].rearrange("p (g u) -> p g u", u=1).broadcast_to((128, gn, K))
                nc.vector.tensor_tensor(
                    out=esub[:, cs].rearrange("p (g k) -> p g k", g=gn),
                    in0=lp3, in1=mb, op=ALU.subtract)
                nc.scalar.activation(e_sb[:, cs], esub[:, cs], AF.Exp)
                nc.vector.tensor_reduce(out=sumt[:, gs],
                                        in_=e_sb[:, cs].rearrange("p (g k) -> p g k", g=gn),
                                        axis=mybir.AxisListType.X, op=ALU.add)
                nc.vector.reciprocal(rcp[:, gs], sumt[:, gs])
                rb = rcp[:, gs].rearrange("p (g u) -> p g u", u=1).broadcast_to((128, gn, K))
                nc.vector.tensor_tensor(out=a_sb[:, cs].rearrange("p (g k) -> p g k", g=gn),
                                        in0=e_sb[:, cs].rearrange("p (g k) -> p g k", g=gn),
                                        in1=rb, op=ALU.mult)

            for s in range(NS):
                ns = slice(s * 512, (s + 1) * 512)
                # --- stem B: y_dn[o, ns] = relu(sum_c wT[c,o]x[c,ns] + bias2[o])
                for o in range(NB):
                    pB = psB.tile([128, 512], F32)
                    for c in range(NB):
                        nc.tensor.matmul(
                            pB[:],
                            wT[:, c, o * 128 : (o + 1) * 128],
                            x_sb[:, c, ns],
                            start=(c == 0),
                            stop=(c == NB - 1),
                        )
                    dst = y_dn[:, o, ns]
                    if s % 2 == 0:
                        nc.scalar.activation(dst, pB[:], AF.Relu, bias=chv[:, o, 0:1])
                    else:
                        if has_bias2:
                            nc.vector.tensor_scalar(
                                out=dst, in0=pB[:], scalar1=chv[:, o, 0:1],
                                scalar2=0.0, op0=ALU.add, op1=ALU.max)
                        else:
                            nc.vector.tensor_scalar_max(out=dst, in0=pB[:], scalar1=0.0)

                # --- stem A: y_nd chunk j via PE transpose of y_dn --------
                # (cheaper than recomputing x^T W': contraction K=128 once,
                #  and bias2/relu already applied by stem B)
                for half in range(2):
                    pA = psA.tile([128, 512], F32R)
                    j0 = 4 * s + 2 * half
                    for ci in range(2):
                        j = j0 + ci
                        jc = slice(j * 128, (j + 1) * 128)
                        for c in range(NB):
                            nc.tensor.transpose(
                                pA[:, (2 * ci + c) * 128 : (2 * ci + c + 1) * 128],
                                y_dn[:, c, jc], ident128)
                    dst = _strided_cols(y_nd, j0 * CW, CW, 2, D)
                    if half == 0:
                        nc.scalar.activation(dst, pA[:], AF.Identity)
                    else:
                        nc.vector.tensor_copy(dst, pA[:])

                # --- squares + logits emitted once a 1024-col quarter of
                #     y_dn is complete (after each odd slice) ------------
                # logits in (k, n) orientation so the 128x32 constants stay
                # (cheaply re-)loaded as stationary weights:
                #   lkn[k, n] = sum_d ct2[d,k] y[d,n] + sum_d srep[d,k] ysq[d,n]
                #             = -2 s_k <y_n, c_k> + s_k |y_n|^2
                # + sc2[k] added as a per-partition bias in the PSUM->SBUF copy,
                # then 32x128 blocks are PE-transposed into (n, k) psum banks.
                if s % 2 == 1:
                    q = s // 2
                    qs = slice(q * 1024, (q + 1) * 1024)
                    for c in range(NB):
                        nc.gpsimd.tensor_mul(ysq[:, c, qs], y_dn[:, c, qs], y_dn[:, c, qs])

                    for si, sl in enumerate((s - 1, s)):
                        pK = psK.tile([32, 512], F32)
                        nsl = slice(sl * 512, (sl + 1) * 512)
                        nc.tensor.matmul(pK[:], ct2[:, 0, :], y_dn[:, 0, nsl],
                                         start=True, stop=False)
                        nc.tensor.matmul(pK[:], ct2[:, 1, :], y_dn[:, 1, nsl],
                                         start=False, stop=False)
                        nc.tensor.matmul(pK[:], srep[:], ysq[:, 0, nsl],
                                         start=False, stop=False)
                        nc.tensor.matmul(pK[:], srep[:], ysq[:, 1, nsl],
                                         start=False, stop=True)
                        dst = lkn[:, nsl]
                        if si == 0:
                            nc.scalar.activation(dst, pK[:], AF.Identity, bias=sc2col[:])
                        else:
                            nc.vector.tensor_scalar_add(out=dst, in0=pK[:],
                                                        scalar1=sc2col[:])
                    for j in range(4 * (s - 1), 4 * s + 4):
                        lp = logits_ps[j // 16]
                        nc.tensor.transpose(lp[:, (j % 16) * 32 : (j % 16) * 32 + 32],
                                            lkn[:, j * 128 : (j + 1) * 128], ident[:])

                # --- softmax over finished logits groups ------------------
                # g0-15 after slice 3 (overlaps the stem second half);
                # g16-23 after slice 5; g24-31 after slice 7 (short tail)
                if s == 3:
                    emit_softmax(0, 16)
                elif s == 5:
                    emit_softmax(16, 8)
                elif s == 7:
                    emit_softmax(24, 8)

        # ---- aggregation: psum (32, 258) = a^T [y | 1 | 1] -------------
        with ExitStack() as tail_ctx:
            psG = tail_ctx.enter_context(tc.tile_pool(name="psG", bufs=1, space="PSUM"))
            psT = tail_ctx.enter_context(tc.tile_pool(name="psT", bufs=2, space="PSUM"))
            psH = tail_ctx.enter_context(tc.tile_pool(name="psH", bufs=2, space="PSUM"))

            pagg = psG.tile([32, CW], F32)
            for g in range(NCH):
                nc.tensor.matmul(
                    pagg[:],
                    a_sb[:, g * K : (g + 1) * K],
                    y_nd[:, g * CW : (g + 1) * CW],
                    start=(g == 0), stop=(g == NCH - 1))

            # agg[k,d] = pagg[k,d] - rowsum_a[k] * centers[k,d]
            rsc = sb.tile([32, D], F32)
            nc.vector.tensor_scalar_mul(out=rsc[:], in0=ckd[:], scalar1=pagg[:, D : D + 1])
            agg_sb = sb.tile([32, D], F32)
            nc.vector.tensor_tensor(out=agg_sb[:], in0=pagg[:, 0:D], in1=rsc[:], op=ALU.subtract)

            # BN1 + relu + mean over k  ->  z per d-block
            z_t = sb.tile([128, NB], F32)
            t_sb = sb.tile([128, NB, K], F32)
            for b in range(NB):
                pT = psT.tile([128, 32], F32)
                nc.tensor.transpose(pT[:], agg_sb[:, b * 128 : (b + 1) * 128], ident[:])
                nc.scalar.activation(t_sb[:, b, :], pT[:], AF.Relu,
                                     bias=chv[:, b, 2:3], scale=chv[:, b, 1:2])
                nc.vector.tensor_reduce(out=z_t[:, b : b + 1],
                                        in_=t_sb[:, b, :],
                                        axis=mybir.AxisListType.X, op=ALU.add)

            # head: gate = 1 + sigmoid(head_w @ z + head_b)
            gate = sb.tile([128, NB], F32)
            eg = sb.tile([128, NB], F32)
            for o in range(NB):
                pH = psH.tile([128, 1], F32)
                for c in range(NB):
                    nc.tensor.matmul(pH[:], hwT[:, c, o * 128 : (o + 1) * 128],
                                     z_t[:, c : c + 1],
                                     start=(c == 0), stop=(c == NB - 1))
                # exp(-(v + head_b)) ; gate = 1 + 1/(1+e)
                nc.scalar.activation(eg[:, o : o + 1], pH[:], AF.Exp,
                                     bias=chv[:, o, 3:4], scale=-1.0)
                nc.vector.tensor_scalar_add(out=eg[:, o : o + 1], in0=eg[:, o : o + 1],
                                            scalar1=1.0)
                nc.vector.reciprocal(gate[:, o : o + 1], eg[:, o : o + 1])
                nc.vector.tensor_scalar_add(out=gate[:, o : o + 1],
                                            in0=gate[:, o : o + 1], scalar1=1.0)

            # gating: out = relu(x * gate[d]) ; stream out per 1024-col block
            for o in range(NB):
                for hh in range(2):
                    cs = slice(hh * 2048, (hh + 1) * 2048)
                    if o == 0:
                        nc.scalar.activation(out_sb[:, o, cs], x_sb[:, o, cs],
                                             AF.Relu, scale=gate[:, o : o + 1])
                    else:
                        nc.vector.tensor_scalar(out=out_sb[:, o, cs], in0=x_sb[:, o, cs],
                                                scalar1=gate[:, o : o + 1], scalar2=0.0,
                                                op0=ALU.mult, op1=ALU.max)
                    qeng[o].dma_start(out_d[o * 128 : (o + 1) * 128, cs], out_sb[:, o, cs])

    nc.compile()
    return nc


_PROGRAM_CACHE = {}


def _get_program(has_bias2):
    key = bool(has_bias2)
    if key not in _PROGRAM_CACHE:
        _PROGRAM_CACHE[key] = _build_program(key)
    return _PROGRAM_CACHE[key]


def _host_params(conv_w, bn2_g, bn2_b, bn2_m, bn2_v, centers, scales,
                 bn1_g, bn1_b, bn1_m, bn1_v, head_w, head_b):
    scale2 = bn2_g / np.sqrt(bn2_v + EPS)
    wT = (conv_w * scale2[:, None]).T.astype(np.float32).copy()      # (c, o)
    bias2 = (bn2_b - bn2_m * scale2).astype(np.float32)
    ct2 = (-2.0 * scales[None, :] * centers.T).astype(np.float32)    # (d, k)
    c2 = (centers * centers).sum(axis=1)
    ssc = scales.reshape(1, K).astype(np.float32)                    # (1, k)
    scc = (scales * c2).reshape(K, 1).astype(np.float32)             # (k, 1)
    s1 = bn1_g / np.sqrt(bn1_v + EPS)
    bb1 = bn1_b - bn1_m * s1
    chv = np.stack([bias2, s1.astype(np.float32), bb1.astype(np.float32),
                    (-head_b).astype(np.float32)], axis=1).astype(np.float32)  # (d, 4)
    hwT = (head_w.T / np.float32(K)).astype(np.float32)              # (d, o)
    identcols = np.zeros((D, 128), np.float32)
    identcols[0:128, :] = np.eye(128, dtype=np.float32)
    packr = np.ascontiguousarray(np.concatenate([wT, ct2, identcols], axis=1))  # (d, 416)
    packf = np.ascontiguousarray(np.concatenate([hwT, chv], axis=1))         # (d, 260)
    small = np.ascontiguousarray(np.concatenate(
        [centers.astype(np.float32), scc], axis=1))                          # (k, 257)
    return packr, packf, small, ssc, bias2


def _ensure_profile_hook():
    """Register the axon NTFF profile hook if the image lacks antenv.axon_hooks."""
    import types

    if "antenv.axon_hooks" in sys.modules:
        return
    try:
        import antenv

        mod = types.ModuleType("antenv.axon_hooks")
        _hook = [None]
        mod.set_axon_ntff_profile_hook = lambda h: _hook.__setitem__(0, h)
        mod.get_axon_ntff_profile_hook = lambda: _hook[0]
        sys.modules["antenv.axon_hooks"] = mod
        antenv.axon_hooks = mod
        from trn_agent_boot.trn_boot import _ntff_profile_via_ctypes

        mod.set_axon_ntff_profile_hook(
            _ntff_profile_via_ctypes("/opt/axon/libaxon_pjrt.so"))
        import concourse.bass_utils as _bu

        _bu.upload_artifacts = lambda d: d  # no artifact store in this container
    except Exception as e:  # profiling is best-effort
        print(f"profile hook setup failed: {e}", file=sys.stderr)


def kernel(x, conv_w, bn2_g, bn2_b, bn2_m, bn2_v, centers, scales,
           bn1_g, bn1_b, bn1_m, bn1_v, head_w, head_b):
    x = np.ascontiguousarray(np.asarray(x, dtype=np.float32))
    packr, packf, small, ssc, bias2 = _host_params(
        np.asarray(conv_w, np.float32), np.asarray(bn2_g, np.float32),
        np.asarray(bn2_b, np.float32), np.asarray(bn2_m, np.float32),
        np.asarray(bn2_v, np.float32), np.asarray(centers, np.float32),
        np.asarray(scales, np.float32), np.asarray(bn1_g, np.float32),
        np.asarray(bn1_b, np.float32), np.asarray(bn1_m, np.float32),
        np.asarray(bn1_v, np.float32), np.asarray(head_w, np.float32),
        np.asarray(head_b, np.float32))
    has_bias2 = bool(np.abs(bias2).max() > 0)
    nc = _get_program(has_bias2)

    shared = {
        "packr": packr, "packf": packf, "small": small, "ssc": ssc,
        "one2": np.ones((1, 2), np.float32),
        "b2r": np.stack([np.ones(D, np.float32), bias2]),
    }
    in_maps = [dict(shared, x=x[b].reshape(D, HW)) for b in range(N_CORES)]

    trace = bool(int(os.environ.get("KERNEL_TRACE", "0")))
    kwargs = {}
    if trace:
        _ensure_profile_hook()
        tdir = os.environ.get("KERNEL_TRACE_DIR")
        if tdir:
            os.makedirs(tdir, exist_ok=True)
            kwargs["tmpdir"] = tdir
    res = run_bass_kernel_spmd(nc, in_maps, list(range(N_CORES)), trace=trace, **kwargs)
    if trace:
        kernel.last_exec_time_ns = res.exec_time_ns
        kernel.last_results = res
    out = np.stack([res.results[b]["out"].reshape(D, H, W) for b in range(N_CORES)])
    return out.astype(np.float32)


# revision 51
# speedup vs baseline: 1.0468x; 1.0468x over previous
"""Trainium2 Bass kernel for nn_EncodingModule2d (vq_codebook).

Pipeline per batch item (pure data parallel, 1 item per NeuronCore, 8 cores):
  stem:   s = conv_w @ x  (1x1 conv as 256x256 matmul over 4096 positions)
          y = relu(BN2(s))                          -- BN folded into weights on host
  vq:     dist2[n,k] = |y_n|^2 - 2<y_n, c_k> + |c_k|^2
          a = softmax_k(scales_k * dist2)
          agg[k,:] = sum_n a[n,k] (y_n - c_k)
  post:   z = mean_k relu(BN1(agg))                 -- BN folded on host
          g = sigmoid(head_w @ z + head_b)
  out:    relu(x + x * g) = relu(x * (1 + g))

The kernel computes the stem in BOTH (d,n) and (n,d) layouts directly from x
(two matmul orientations) because the distance matmul contracts over d while
the aggregation matmul contracts over n; this costs the same PE time as one
stem plus PE transposes but avoids ~20us of PSUM->SBUF copy traffic.

dtype strategy: float32r (1 cyc/row on the PE when N>=256, vs 4 for float32)
for the stem and aggregation matmuls; plain float32 for the N=32 distance
matmuls where fp32r has no speed advantage anyway (and would force rounding
of y). fp32r matmuls require even N, hence the 258-wide aggregation rhs
(256 y columns + ones column + dummy pad column).
"""

import os
import sys

for _p in ("/opt/trn_rl_repo",):
    if _p not in sys.path and os.path.isdir(_p):
        sys.path.insert(0, _p)

from contextlib import ExitStack

import numpy as np

import concourse.bass as bass
import concourse.tile as tile
from concourse import bacc, mybir
from concourse.bass_utils import run_bass_kernel_spmd
from concourse.masks import make_identity

F32 = mybir.dt.float32
F32R = mybir.dt.float32r
AF = mybir.ActivationFunctionType
ALU = mybir.AluOpType

B, D, H, W, K = 8, 256, 64, 64, 32
HW = H * W          # 4096 spatial positions
NB = D // 128       # 2 channel blocks of 128
NS = HW // 512      # 8 n-slices of 512
NCH = HW // 128     # 32 n-chunks of 128
CW = D + 2          # y_nd chunk width: 256 y + ones + pad (fp32r needs even N)
EPS = 1e-5
N_CORES = 8


def _strided_cols(t, start, step, count, width):
    """AP over columns [start + i*step : start + i*step + width) of a 2D tile."""
    a = t[:, start : start + 1]
    return bass.AP(tensor=a.tensor, offset=a.offset, ap=[a.ap[0], [step, count], [1, width]])


def _build_program(has_bias2):
    nc = bacc.Bacc("TRN2", target_bir_lowering=False, debug=False, num_devices=N_CORES)

    x_d = nc.dram_tensor("x", [D, HW], F32R, kind="ExternalInput").ap()
    # packed constants: fewer DMA triggers (each costs ~0.7us of SEQ time)
    # packr: [wT | ct2 | identity(128) in c-block 0 rows]
    pr_d = nc.dram_tensor("packr", [D, D + K + 128], F32R, kind="ExternalInput").ap()
    pf_d = nc.dram_tensor("packf", [D, D + 4], F32, kind="ExternalInput").ap()    # [hwT | chv]
    # rows 0..31: [centers | scales*|c|^2 | pad]; row 32: [-head_b | pad]
    sm_d = nc.dram_tensor("small", [K + 1, D + 2], F32, kind="ExternalInput").ap()
    ssc_d = nc.dram_tensor("ssc", [1, K], F32R, kind="ExternalInput").ap()
    one2_d = nc.dram_tensor("one2", [1, 2], F32R, kind="ExternalInput").ap()
    b2r_d = nc.dram_tensor("b2r", [2, D], F32R, kind="ExternalInput").ap()  # [ones, bias2]
    out_d = nc.dram_tensor("out", [D, HW], F32, kind="ExternalOutput").ap()

    x3 = x_d.rearrange("(c p) n -> p c n", p=128)

    with tile.TileContext(nc) as tc, ExitStack() as ctx:
        sb = ctx.enter_context(tc.tile_pool(name="sb", bufs=1))

        # ---- loads: first x slice, then weights, then the rest ---------
        x_sb = sb.tile([128, NB, HW], F32R)
        packr = sb.tile([128, NB, D + K + 128], F32R)
        packf = sb.tile([128, NB, D + 4], F32)
        small = sb.tile([K + 1, D + 2], F32)
        srep = sb.tile([128, K], F32R)           # scales replicated over partitions
        one2 = sb.tile([128, 2], F32R)           # fp32r ones (memset can't do f32r)
        b2row = sb.tile([2, D], F32R)            # [ones row, bias2 row] (stem A bias)

        # x: contiguous per-c-block pieces, c0 on the sync queue and c1 on
        # the scalar queue so both flow in parallel and slice 0 lands early
        qeng = [nc.sync, nc.scalar]
        pieces = [(0, 512), (512, 1536), (1536, 2560), (2560, 4096)]
        for q, (lo, hi) in enumerate(pieces):
            cs = slice(lo, hi)
            for c in range(NB):
                qeng[c].dma_start(x_sb[:, c, cs], x_d[c * 128 : (c + 1) * 128, cs])
            if q == 0:
                nc.sync.dma_start(packr[:], pr_d.rearrange("(c p) m -> p c m", p=128))
        nc.scalar.dma_start(packf[:], pf_d.rearrange("(c p) m -> p c m", p=128))
        nc.sync.dma_start(small[:], sm_d)
        nc.scalar.dma_start(srep[:], ssc_d.partition_broadcast(128))
        nc.sync.dma_start(one2[:], one2_d.partition_broadcast(128))
        if has_bias2:
            nc.sync.dma_start(b2row[:], b2r_d)

        wT = packr[:, :, 0:D]                    # wT[c,:,o] per c-block
        ct2 = packr[:, :, D : D + K]             # -2*scales[k]*centers[k,d]
        ident128 = packr[:, 0, D + K : D + K + 128]   # 128x128 identity (f32r)
        hwT = packf[:, :, 0:D]                   # head_w.T / K
        chv = packf[:, :, D : D + 4]             # [bias2, s1, bb1, -head_b]
        ckd = small[0:K, 0:D]                    # centers (k,d)
        sc2col = small[0:K, D : D + 1]           # scales[k]*|c_k|^2 (bias column)
        nhrow = small[K : K + 1, 0:D]            # +head_b as a row

        ident = sb.tile([32, 32], F32)
        make_identity(nc, ident[:])

        # warm the exp table on ACT early (hidden under the x DMA)
        warm = sb.tile([128, 1], F32)
        nc.vector.memset(warm[:], 0.0)
        nc.scalar.activation(warm[:], warm[:], AF.Exp)

        # ---- big intermediates ----------------------------------------
        y_dn = sb.tile([128, NB, HW], F32R)      # relu(W'x): d on partitions
        y_nd = sb.tile([128, NCH * CW], F32R)    # per chunk: 256 y cols + [1, 1]
        ysq = sb.tile([128, NB, HW], F32R)       # y_dn^2
        lkn = sb.tile([32, HW], F32)             # logits in (k, n) layout
        esub = sb.tile([128, NCH * K], F32)      # logits - max
        e_sb = sb.tile([128, NCH * K], F32)      # exp(...)
        a_sb = sb.tile([128, NCH * K], F32R)     # softmax weights
        out_sb = sb.tile([128, NB, HW], F32)

        # ones + pad columns of y_nd (DVE copy from f32r const)
        nc.vector.tensor_copy(
            _strided_cols(y_nd, D, CW, NCH, 2),
            one2[:].rearrange("p (u k) -> p u k", u=1).broadcast_to((128, NCH, 2)))

        with ExitStack() as stem_ctx:
            psB = stem_ctx.enter_context(tc.tile_pool(name="psB", bufs=2, space="PSUM"))
            psA = stem_ctx.enter_context(tc.tile_pool(name="psA", bufs=2, space="PSUM"))
            psK = stem_ctx.enter_context(tc.tile_pool(name="psK", bufs=2, space="PSUM"))
            psL = stem_ctx.enter_context(tc.tile_pool(name="psL", bufs=1, space="PSUM"))

            logits_ps = [psL.tile([128, 512], F32, name=f"logits{i}") for i in range(2)]
            maxt = sb.tile([128, NCH], F32)
            sumt = sb.tile([128, NCH], F32)
            rcp = sb.tile([128, NCH], F32)

            # HAM warm-up: keep the PE busy on dummy transposes of the small
            # identity while x streams in, so the clock gate is at 8/8 when
            # the real matmuls start. Results land in a logits_ps corner that
            # the real chunk transposes later overwrite (start=True).
            for i in range(32):
                nc.tensor.transpose(logits_ps[i % 2][0:32, 128:160],
                                    ident[:], ident[:])
            # larger f32r dummies bridge until the first x piece lands
            for i in range(12):
                pW = psA.tile([128, 512], F32R, name="warm", tag="pA")
                for u in range(2):
                    nc.tensor.transpose(pW[:, u * 128 : (u + 1) * 128],
                                        packr[:, 0, 0:128], ident128)

            def emit_softmax(g0, gn):
                t = g0 // 16
                gs = slice(g0, g0 + gn)
                cs = slice(g0 * K, (g0 + gn) * K)
                lcs = slice((g0 % 16) * K, ((g0 % 16) + gn) * K)
                lp3 = logits_ps[t][:, lcs].rearrange("p (g k) -> p g k", g=gn)
                nc.vector.tensor_reduce(out=maxt[:, gs], in_=lp3,
                                        axis=mybir.AxisListType.X, op=ALU.max)
                mb = maxt[:, gs].rearrange("p (g u) -> p g u", u=1).broadcast_to((128, gn, K))
                nc.vector.tensor_tensor(
                    out=esub[:, cs].rearrange("p (g k) -> p g k", g=gn),
                    in0=lp3, in1=mb, op=ALU.subtract)
                nc.scalar.activation(e_sb[:, cs], esub[:, cs], AF.Exp)
                nc.vector.tensor_reduce(out=sumt[:, gs],
                                        in_=e_sb[:, cs].rearrange("p (g k) -> p g k", g=gn),
                                        axis=mybir.AxisListType.X, op=ALU.add)
                nc.vector.reciprocal(rcp[:, gs], sumt[:, gs])
                rb = rcp[:, gs].rearrange("p (g u) -> p g u", u=1).broadcast_to((128, gn, K))
                nc.vector.tensor_tensor(out=a_sb[:, cs].rearrange("p (g k) -> p g k", g=gn),
                                        in0=e_sb[:, cs].rearrange("p (g k) -> p g k", g=gn),
                                        in1=rb, op=ALU.mult)

            for s in range(NS):
                ns = slice(s * 512, (s + 1) * 512)
                # --- stem B: y_dn[o, ns] = relu(sum_c wT[c,o]x[c,ns] + bias2[o])
                for o in range(NB):
                    pB = psB.tile([128, 512], F32)
                    for c in range(NB):
                        nc.tensor.matmul(
                            pB[:],
                            wT[:, c, o * 128 : (o + 1) * 128],
                            x_sb[:, c, ns],
                            start=(c == 0),
                            stop=(c == NB - 1),
                        )
                    dst = y_dn[:, o, ns]
                    if s % 2 == 0:
                        nc.scalar.activation(dst, pB[:], AF.Relu, bias=chv[:, o, 0:1])
                    else:
                        if has_bias2:
                            nc.vector.tensor_scalar(
                                out=dst, in0=pB[:], scalar1=chv[:, o, 0:1],
                                scalar2=0.0, op0=ALU.add, op1=ALU.max)
                        else:
                            nc.vector.tensor_scalar_max(out=dst, in0=pB[:], scalar1=0.0)

                # --- stem A: y_nd chunk j via PE transpose of y_dn --------
                # (cheaper than recomputing x^T W': contraction K=128 once,
                #  and bias2/relu already applied by stem B)
                for half in range(2):
                    pA = psA.tile([128, 512], F32R)
                    j0 = 4 * s + 2 * half
                    for ci in range(2):
                        j = j0 + ci
                        jc = slice(j * 128, (j + 1) * 128)
                        for c in range(NB):
                            nc.tensor.transpose(
                                pA[:, (2 * ci + c) * 128 : (2 * ci + c + 1) * 128],
                                y_dn[:, c, jc], ident128)
                    dst = _strided_cols(y_nd, j0 * CW, CW, 2, D)
                    if half == 0:
                        nc.scalar.activation(dst, pA[:], AF.Identity)
                    else:
                        nc.vector.tensor_copy(dst, pA[:])

                # --- squares + logits emitted once a 1024-col quarter of
                #     y_dn is complete (after each odd slice) ------------
                # logits in (k, n) orientation so the 128x32 constants stay
                # (cheaply re-)loaded as stationary weights:
                #   lkn[k, n] = sum_d ct2[d,k] y[d,n] + sum_d srep[d,k] ysq[d,n]
                #             = -2 s_k <y_n, c_k> + s_k |y_n|^2
                # + sc2[k] added as a per-partition bias in the PSUM->SBUF copy,
                # then 32x128 blocks are PE-transposed into (n, k) psum banks.
                if s % 2 == 1:
                    q = s // 2
                    qs = slice(q * 1024, (q + 1) * 1024)
                    for c in range(NB):
                        nc.gpsimd.tensor_mul(ysq[:, c, qs], y_dn[:, c, qs], y_dn[:, c, qs])

                    for si, sl in enumerate((s - 1, s)):
                        pK = psK.tile([32, 512], F32)
                        nsl = slice(sl * 512, (sl + 1) * 512)
                        nc.tensor.matmul(pK[:], ct2[:, 0, :], y_dn[:, 0, nsl],
                                         start=True, stop=False)
                        nc.tensor.matmul(pK[:], ct2[:, 1, :], y_dn[:, 1, nsl],
                                         start=False, stop=False)
                        nc.tensor.matmul(pK[:], srep[:], ysq[:, 0, nsl],
                                         start=False, stop=False)
                        nc.tensor.matmul(pK[:], srep[:], ysq[:, 1, nsl],
                                         start=False, stop=True)
                        dst = lkn[:, nsl]
                        if si == 0:
                            nc.scalar.activation(dst, pK[:], AF.Identity, bias=sc2col[:])
                        else:
                            nc.vector.tensor_scalar_add(out=dst, in0=pK[:],
                                                        scalar1=sc2col[:])
                    for j in range(4 * (s - 1), 4 * s + 4):
                        lp = logits_ps[j // 16]
                        nc.tensor.transpose(lp[:, (j % 16) * 32 : (j % 16) * 32 + 32],
                                            lkn[:, j * 128 : (j + 1) * 128], ident[:])

                # --- softmax over finished logits groups ------------------
                # g0-15 after slice 3 (overlaps the stem second half);
                # g16-23 after slice 5; g24-31 after slice 7 (short tail)
                if s == 3:
                    emit_softmax(0, 16)
                elif s == 5:
                    emit_softmax(16, 8)
                elif s == 7:
                    emit_softmax(24, 8)

        # ---- aggregation: psum (32, 258) = a^T [y | 1 | 1] -------------
        with ExitStack() as tail_ctx:
            psG = tail_ctx.enter_context(tc.tile_pool(name="psG", bufs=1, space="PSUM"))
            psT = tail_ctx.enter_context(tc.tile_pool(name="psT", bufs=2, space="PSUM"))
            psH = tail_ctx.enter_context(tc.tile_pool(name="psH", bufs=2, space="PSUM"))

            # keep the PE clock warm while the last softmax segment runs
            for i in range(10):
                pWt = psT.tile([128, 32], F32, name="warmt", tag="pT")
                nc.tensor.transpose(pWt[:], lkn[:, 0:128], ident[:])

            pagg = psG.tile([32, CW], F32)
            for g in range(NCH):
                nc.tensor.matmul(
                    pagg[:],
                    a_sb[:, g * K : (g + 1) * K],
                    y_nd[:, g * CW : (g + 1) * CW],
                    start=(g == 0), stop=(g == NCH - 1))

            # agg[k,d] = pagg[k,d] - rowsum_a[k] * centers[k,d]
            rsc = sb.tile([32, D], F32)
            nc.vector.tensor_scalar_mul(out=rsc[:], in0=ckd[:], scalar1=pagg[:, D : D + 1])
            agg_sb = sb.tile([32, D], F32)
            nc.vector.tensor_tensor(out=agg_sb[:], in0=pagg[:, 0:D], in1=rsc[:], op=ALU.subtract)

            # BN1 + relu + mean over k  ->  z per d-block
            z_t = sb.tile([128, NB], F32)
            t_sb = sb.tile([128, NB, K], F32)
            for b in range(NB):
                pT = psT.tile([128, 32], F32)
                nc.tensor.transpose(pT[:], agg_sb[:, b * 128 : (b + 1) * 128], ident[:])
                nc.scalar.activation(t_sb[:, b, :], pT[:], AF.Relu,
                                     bias=chv[:, b, 2:3], scale=chv[:, b, 1:2])
                nc.vector.tensor_reduce(out=z_t[:, b : b + 1],
                                        in_=t_sb[:, b, :],
                                        axis=mybir.AxisListType.X, op=ALU.add)

            # head: gate = 1 + sigmoid(head_w @ z + head_b)
            gate = sb.tile([128, NB], F32)
            eg = sb.tile([128, NB], F32)
            for o in range(NB):
                pH = psH.tile([128, 1], F32)
                for c in range(NB):
                    nc.tensor.matmul(pH[:], hwT[:, c, o * 128 : (o + 1) * 128],
                                     z_t[:, c : c + 1],
                                     start=(c == 0), stop=(c == NB - 1))
                # exp(-(v + head_b)) ; gate = 1 + 1/(1+e)
                nc.scalar.activation(eg[:, o : o + 1], pH[:], AF.Exp,
                                     bias=chv[:, o, 3:4], scale=-1.0)
            nc.vector.tensor_scalar_add(out=eg[:], in0=eg[:], scalar1=1.0)
            nc.vector.reciprocal(gate[:], eg[:])
            nc.vector.tensor_scalar_add(out=gate[:], in0=gate[:], scalar1=1.0)

            # gating: out = relu(x * gate[d]) ; stream out per 1024-col block
            for o in range(NB):
                for hh in range(2):
                    cs = slice(hh * 2048, (hh + 1) * 2048)
                    if o == 0:
                        nc.scalar.activation(out_sb[:, o, cs], x_sb[:, o, cs],
                                             AF.Relu, scale=gate[:, o : o + 1])
                    else:
                        nc.vector.tensor_scalar(out=out_sb[:, o, cs], in0=x_sb[:, o, cs],
                                                scalar1=gate[:, o : o + 1], scalar2=0.0,
                                                op0=ALU.mult, op1=ALU.max)
                    qeng[o].dma_start(out_d[o * 128 : (o + 1) * 128, cs], out_sb[:, o, cs])

    nc.compile()
    return nc


_PROGRAM_CACHE = {}


def _get_program(has_bias2):
    key = bool(has_bias2)
    if key not in _PROGRAM_CACHE:
        _PROGRAM_CACHE[key] = _build_program(key)
    return _PROGRAM_CACHE[key]


def _host_params(conv_w, bn2_g, bn2_b, bn2_m, bn2_v, centers, scales,
                 bn1_g, bn1_b, bn1_m, bn1_v, head_w, head_b):
    scale2 = bn2_g / np.sqrt(bn2_v + EPS)
    wT = (conv_w * scale2[:, None]).T.astype(np.float32).copy()      # (c, o)
    bias2 = (bn2_b - bn2_m * scale2).astype(np.float32)
    ct2 = (-2.0 * scales[None, :] * centers.T).astype(np.float32)    # (d, k)
    c2 = (centers * centers).sum(axis=1)
    ssc = scales.reshape(1, K).astype(np.float32)                    # (1, k)
    scc = (scales * c2).reshape(K, 1).astype(np.float32)             # (k, 1)
    s1 = bn1_g / np.sqrt(bn1_v + EPS)
    bb1 = bn1_b - bn1_m * s1
    chv = np.stack([bias2, s1.astype(np.float32), bb1.astype(np.float32),
                    (-head_b).astype(np.float32)], axis=1).astype(np.float32)  # (d, 4)
    hwT = (head_w.T / np.float32(K)).astype(np.float32)              # (d, o)
    identcols = np.zeros((D, 128), np.float32)
    identcols[0:128, :] = np.eye(128, dtype=np.float32)
    packr = np.ascontiguousarray(np.concatenate([wT, ct2, identcols], axis=1))  # (d, 416)
    packf = np.ascontiguousarray(np.concatenate([hwT, chv], axis=1))         # (d, 260)
    small = np.zeros((K + 1, D + 2), np.float32)
    small[0:K, 0:D] = centers
    small[0:K, D] = scc[:, 0]
    small[K, 0:D] = head_b
    small[K, D] = 1.0
    return packr, packf, small, ssc, bias2


def _ensure_profile_hook():
    """Register the axon NTFF profile hook if the image lacks antenv.axon_hooks."""
    import types

    if "antenv.axon_hooks" in sys.modules:
        return
    try:
        import antenv

        mod = types.ModuleType("antenv.axon_hooks")
        _hook = [None]
        mod.set_axon_ntff_profile_hook = lambda h: _hook.__setitem__(0, h)
        mod.get_axon_ntff_profile_hook = lambda: _hook[0]
        sys.modules["antenv.axon_hooks"] = mod
        antenv.axon_hooks = mod
        from trn_agent_boot.trn_boot import _ntff_profile_via_ctypes

        mod.set_axon_ntff_profile_hook(
            _ntff_profile_via_ctypes("/opt/axon/libaxon_pjrt.so"))
        import concourse.bass_utils as _bu

        _bu.upload_artifacts = lambda d: d  # no artifact store in this container
    except Exception as e:  # profiling is best-effort
        print(f"profile hook setup failed: {e}", file=sys.stderr)


def kernel(x, conv_w, bn2_g, bn2_b, bn2_m, bn2_v, centers, scales,
           bn1_g, bn1_b, bn1_m, bn1_v, head_w, head_b):
    x = np.ascontiguousarray(np.asarray(x, dtype=np.float32))
    packr, packf, small, ssc, bias2 = _host_params(
        np.asarray(conv_w, np.float32), np.asarray(bn2_g, np.float32),
        np.asarray(bn2_b, np.float32), np.asarray(bn2_m, np.float32),
        np.asarray(bn2_v, np.float32), np.asarray(centers, np.float32),
        np.asarray(scales, np.float32), np.asarray(bn1_g, np.float32),
        np.asarray(bn1_b, np.float32), np.asarray(bn1_m, np.float32),
        np.asarray(bn1_v, np.float32), np.asarray(head_w, np.float32),
        np.asarray(head_b, np.float32))
    has_bias2 = bool(np.abs(bias2).max() > 0)
    nc = _get_program(has_bias2)

    shared = {
        "packr": packr, "packf": packf, "small": small, "ssc": ssc,
        "one2": np.ones((1, 2), np.float32),
        "b2r": np.stack([np.ones(D, np.float32), bias2]),
    }
    in_maps = [dict(shared, x=x[b].reshape(D, HW)) for b in range(N_CORES)]

    trace = bool(int(os.environ.get("KERNEL_TRACE", "0")))
    kwargs = {}
    if trace:
        _ensure_profile_hook()
        tdir = os.environ.get("KERNEL_TRACE_DIR")
        if tdir:
            os.makedirs(tdir, exist_ok=True)
            kwargs["tmpdir"] = tdir
    res = run_bass_kernel_spmd(nc, in_maps, list(range(N_CORES)), trace=trace, **kwargs)
    if trace:
        kernel.last_exec_time_ns = res.exec_time_ns
        kernel.last_results = res
    out = np.stack([res.results[b]["out"].reshape(D, H, W) for b in range(N_CORES)])
    return out.astype(np.float32)
